# revision 67
# baseline (speedup 1.0000x reference)
"""Trainium2 Bass kernel for a track-wise (ragged-sequence) attention layer.

Math (per track t of length L, per head h):
    qkv = values @ w_qkv.T + b_qkv                      # [N, 3*256]
    S   = q k^T / sqrt(Dh);  P = softmax(S);  ctx = P v
    out = ctx @ w_lin.T + b_lin

Device strategy (data parallel over tracks, 8 cores, no cross-core comm):
  - scores are computed directly transposed per head: ST = K @ Q^T, so
    exp(ST) is exactly the lhs^T the ctx matmul needs -- no PE transposes.
  - no max-subtraction in softmax (scores are ~N(0,1); exp is safe in fp32).
  - the softmax denominator is folded into the ctx matmul: lhsT = [V_h | 1],
    so one PE pass yields [ctx^T; rowsum] in PSUM (a single ones column;
    32 ones columns only for the last two iterations, see tail below).
  - RECIPROCAL DANCE: scalar-engine activation cost is free-size-per-lane,
    so the old [32,4,256]-layout ln/exp reciprocal cost 2.2us/iter for 1024
    unique values.  Now: one DVE copy drains [ctx^T; rowsum] PSUM->SBUF
    (bf16), per iteration-PAIR one SP-queue DMA reshapes the 2048 rowsums
    [1,2048]->[128,16], scalar ln+exp runs at 16 elem/lane (~0.5us/pair),
    an Act-queue DMA writes a DRAM slot, and an SP-queue DMA broadcasts
    back to [32,2,4,256].  Scalar recip cost: 139us -> ~15us total.
  - the normalize multiply runs on the otherwise-idle GPSIMD/Pool engine
    (SBUF x SBUF); the last 4 iterations split it Pool/DVE, and the last 2
    use a direct path (32 ones-columns + in-place scalar ln/exp + DVE mult
    from PSUM) because the scalar engine is idle once score exps end.
  - bf16 for et4/v_aug/ctx^T/cacc/rcp4/w_lin: same 1 cyc/row PE rate,
    halves SBUF, funds the lag-5 software pipeline (norm mult of iter j
    lands at slot j+5).
  - PSUM pooling: ST tiles (4 banks, depth 2), the ctx/rowsum accumulator
    (2 banks, single-buffered -- the DVE copy drains it within a slot), and
    1-bank tiles for the a1/a2/c projections (2 banks).  Separating the
    projection tiles from the ST pool stops the PE stalling on exp's PSUM
    reads (that stall was resetting the PE DVFS state every iteration:
    mid-clock 1.2GHz costs 2x cycles until 3us of continuous busy).
  - ST matmuls issue tile-major with alternating banks so the first exp's
    tile completes after 4 matmuls, not 6 (never >1 consecutive write to
    the same PSUM bank -- consecutive same-bank writes serialize).
  - DMA queue discipline: xg loads + dance T1/T3 + output stores on the SP
    queue, dance T2 on the Act queue right after the exp that feeds it
    (zero blocking wait), consts/memset on Pool.  Queue assignment matters
    more than engine busy: a blocked DMA trigger stalls its whole in-order
    queue (that serialization cost ~60us in earlier revisions).
  - weave: the next group's projections and the previous group's output
    units interleave through the attention iterations; c_unit[ci] is placed
    past slot (2*ci + MULT_LAG - 7 + C_MARGIN) where its norm mults have
    executed.
  - 1/sqrt(Dh) is folded into w_q/b_q on the host; b_v is folded into the
    final bias (softmax rows sum to 1): b_final = w_lin @ b_v + b_lin.
  - fp32 matmuls run as float32r (full rate with >=256 free out).

History: 812us (first session) -> 354us (second session, scalar-bound:
276us scalar of which 139us was the reciprocal) -> 242-247us (this
session).  Engine busy now ~PE 210 (DVFS-mixed), Pool 170, DVE 170,
scalar 155us.  Measured dead ends this session: custom-DVE ops and
gpsimd.partition_broadcast ("ISA wrong length" in this walrus build),
ldw-opt with 33-col bf16 lhsT, CTX_LAG=2 exp decoupling, readiness-greedy
weave, xg loads issued a group early (DMA collisions with the dance),
C_MARGIN=2 (327us!), fp8 needs DoubleRow layout gymnastics to beat bf16.
"""

import os
import sys

import numpy as np

for _p in ("/opt/trn_rl_repo", "/root/.axon_site/_ro/trn_rl_repo"):
    if os.path.isdir(_p) and _p not in sys.path:
        sys.path.append(_p)

import ml_dtypes

import concourse.bass as bass
import concourse.tile as tile
from concourse import mybir
from concourse.bass_utils import run_bass_kernel_spmd

F32 = mybir.dt.float32
F32R = mybir.dt.float32r
BF16 = mybir.dt.bfloat16
EXP = mybir.ActivationFunctionType.Exp
LOG = mybir.ActivationFunctionType.Ln
IDENT = mybir.ActivationFunctionType.Identity
MUL = mybir.AluOpType.mult
ADD = mybir.AluOpType.add

N_CORES = 8
N, DIN, DOUT, H, T, L = 65536, 256, 256, 8, 256, 256

DH = DOUT // H          # 32
PC = N // N_CORES       # 8192 points per core
TPC = T // N_CORES      # 32 tracks per core
TPG = 4                 # tracks per group
NG = TPC // TPG         # 8 groups
GP = TPG * L            # 1024 points per group
MC_G = GP // 128        # 8 128-point chunks per group
NIT = NG * TPG * 2      # 64 iterations total

# tuning knobs
MULT_LAG = int(os.environ.get("MULT_LAG", "5"))
CTX_LAG = int(os.environ.get("CTX_LAG", "1"))   # slots between ST and ctx
T3_SLOT = int(os.environ.get("T3_SLOT", "1"))   # T3 delay after T2 (slots)
ST_ORDER = os.environ.get("ST_ORDER", "tile")   # j | tile matmul order
TAIL_T3 = int(os.environ.get("TAIL_T3", "0"))   # last pairs: same-slot T3
C_MARGIN = int(os.environ.get("C_MARGIN", "3"))
XG_EARLY = os.environ.get("XG_EARLY", "0") == "1"
PRE0 = os.environ.get("PRE0", "1") == "1"
A_DELAY = float(os.environ.get("A_DELAY", "1.5"))  # xg DMA latency, slots
XG_Q = os.environ.get("XG_Q", "sync")   # queue for xg load triggers
MULT_DVE = int(os.environ.get("MULT_DVE", "0"))  # of 4 norm-mults on DVE
MULT_OP = os.environ.get("MULT_OP", "tt")        # tt | stt pool ucode
TAIL_DVE = int(os.environ.get("TAIL_DVE", "4"))  # last iters: mults on DVE
TAIL_DIRECT = os.environ.get("TAIL_DIRECT", "1") == "1"  # last 2 iters:
# 32 ones-columns + direct scalar ln/exp + DVE mult (no DMA dance)
A1_SCALAR = int(os.environ.get("A1_SCALAR", "0"))   # of 4 rr units on scalar
DANCE_SYNC = os.environ.get("DANCE_SYNC", "1") == "1"  # dance DMAs on SP


class _TileContext(tile.TileContext):
    """TileContext whose final drain carries at most one semaphore wait per
    instruction (the walrus build in this container rejects multi-wait
    TPB_CTRL instructions)."""

    def _drain_and_barrier(self, tick_clock, wait_clock):
        super()._drain_and_barrier(tick_clock, wait_clock)
        self._split_multi_waits()

    def _split_multi_waits(self):
        nc = self.nc
        for f in nc.m.functions:
            for bb in f.blocks:
                changed = False
                new_insts = []
                for inst in bb.instructions:
                    si = inst.sync_info
                    if si is not None and len(si.on_wait) > 1:
                        waits = list(si.on_wait)
                        for w in waits[:-1]:
                            nop = mybir.InstNoOp(
                                name=f"I-{nc.next_id()}", ins=[], outs=[]
                            )
                            nop.engine = inst.engine
                            nop.sync_info = mybir.SyncInfo(
                                on_wait=[w], on_update=[]
                            )
                            new_insts.append(nop)
                        inst.sync_info = mybir.SyncInfo(
                            on_wait=[waits[-1]], on_update=list(si.on_update)
                        )
                        changed = True
                    new_insts.append(inst)
                if changed:
                    bb.instructions = new_insts


def _ensure_ntff_hook():
    """The agent image's ``antenv`` lacks ``axon_hooks``; provide it so
    ``run_bass_kernel_spmd(trace=True)`` can register the axon NTFF hook."""
    try:
        import antenv.axon_hooks  # noqa: F401
        return
    except ImportError:
        pass
    import types

    import antenv

    mod = types.ModuleType("antenv.axon_hooks")
    _hook = [None]
    mod.set_axon_ntff_profile_hook = lambda h: _hook.__setitem__(0, h)
    mod.get_axon_ntff_profile_hook = lambda: _hook[0]
    sys.modules["antenv.axon_hooks"] = mod
    antenv.axon_hooks = mod
    try:
        from trn_agent_boot.trn_boot import _ntff_profile_via_ctypes

        mod.set_axon_ntff_profile_hook(
            _ntff_profile_via_ctypes("/opt/axon/libaxon_pjrt.so")
        )
    except Exception as e:  # pragma: no cover - tracing is best-effort
        print(f"ntff hook setup failed: {e}", file=sys.stderr)


def _build_program(masked: bool):
    nc = bass.Bass("TRN2", target_bir_lowering=False, debug=False,
                   num_devices=N_CORES)

    xT = nc.dram_tensor("xT", [2, 128, PC], F32R, kind="ExternalInput").ap()
    wqk = nc.dram_tensor("wqk", [2, 128, 512], F32R, kind="ExternalInput").ap()
    wv = nc.dram_tensor("wv", [2, 128, 256], F32R, kind="ExternalInput").ap()
    wl = nc.dram_tensor("wl", [2, 128, 256], BF16, kind="ExternalInput").ap()
    bqk = nc.dram_tensor("bqk", [128, 4], F32, kind="ExternalInput").ap()
    bfin = nc.dram_tensor("bfin", [256], F32, kind="ExternalInput").ap()
    rdz = nc.dram_tensor("rdz", [NIT // 2, 2048], BF16, kind="Internal").ap()
    if masked:
        mkf = nc.dram_tensor("mkf", [PC], F32, kind="ExternalInput").ap()
        mkp = nc.dram_tensor("mkp", [128, PC // 128], F32,
                             kind="ExternalInput").ap()
    out = nc.dram_tensor("out", [PC, DOUT], F32, kind="ExternalOutput").ap()

    dq = nc.sync if DANCE_SYNC else nc.gpsimd

    with _TileContext(nc) as tc:
        with (
            tc.tile_pool(name="consts", bufs=1) as consts,
            tc.tile_pool(name="xg", bufs=3) as xg_pool,
            tc.tile_pool(name="qk", bufs=2) as qk_pool,
            tc.tile_pool(name="vsb", bufs=3) as v_pool,
            tc.tile_pool(name="et", bufs=5) as et_pool,
            tc.tile_pool(name="ctxT", bufs=3) as ctx_pool,
            tc.tile_pool(name="osb", bufs=2) as o_pool,
            tc.tile_pool(name="cacc", bufs=(MULT_LAG + 5) // 2) as cacc_pool,
            tc.tile_pool(name="zd", bufs=3) as z_pool,
            tc.tile_pool(name="rcp4", bufs=4) as rcp4_pool,
            tc.tile_pool(name="ps_st", bufs=2, space="PSUM") as ps_st,
            tc.tile_pool(name="ps_acc", bufs=1, space="PSUM") as ps_acc,
            tc.tile_pool(name="ps_misc", bufs=2, space="PSUM") as ps_misc,
        ):
            # ---- constants -------------------------------------------------
            # only wqk/bqk gate the first projection; defer the other
            # const loads until after group 0's input is on the queue
            wqk_sb = consts.tile([128, 2, 512], F32R)
            nc.gpsimd.dma_start(out=wqk_sb, in_=wqk.rearrange("k p r -> p k r"))
            bqk_sb = consts.tile([128, 4], F32)
            nc.gpsimd.dma_start(out=bqk_sb, in_=bqk)
            wv_sb = consts.tile([128, 2, 256], F32R)
            wl_sb = consts.tile([128, 2, 256], BF16)
            bfin_sb = consts.tile([128, 256], F32)

            def load_late_consts():
                nc.gpsimd.dma_start(out=wv_sb,
                                    in_=wv.rearrange("k p r -> p k r"))
                nc.gpsimd.dma_start(out=wl_sb,
                                    in_=wl.rearrange("k p r -> p k r"))
                nc.gpsimd.dma_start(out=bfin_sb,
                                    in_=bfin.partition_broadcast(128))
            if masked:
                mkp_sb = consts.tile([128, PC // 128], F32)
                nc.gpsimd.dma_start(out=mkp_sb, in_=mkp)

            def alloc_xg(g):
                xg = xg_pool.tile([128, 2, GP], F32R, tag="xg", name=f"xg{g}")
                mk_sb = None
                if masked:
                    gsl = slice(g * GP, (g + 1) * GP)
                    mk_sb = et_pool.tile([128, GP], F32, tag="mk",
                                         name=f"mk{g}", bufs=2)
                    nc.gpsimd.dma_start(
                        out=mk_sb, in_=mkf[gsl].partition_broadcast(128)
                    )
                return xg, mk_sb

            def load_xg_chunk(xgmk, g, ci, nch):
                """One 1/nch slice of the group input (no upstream deps:
                DRAM src, long-free tile -- never blocks its queue)."""
                xg, _ = xgmk
                w = GP // nch
                hsl = slice(g * GP + ci * w, g * GP + (ci + 1) * w)
                eng = {"sync": nc.sync, "scalar": nc.scalar,
                       "gpsimd": nc.gpsimd}[XG_Q]
                eng.dma_start(
                    out=xg[:, :, ci * w:(ci + 1) * w],
                    in_=xT[:, :, hsl].rearrange("k p n -> p k n"),
                )

            def emit_load(g):
                xgmk = alloc_xg(g)
                for hf in range(2):
                    load_xg_chunk(xgmk, g, hf, 2)
                return xgmk

            def emit_a_units(g, xg, mk_sb):
                """Return (qk_sb, v_aug, units): units are small emission
                thunks (2 MMs + 1 DVE/scalar op each) to weave between B
                iterations so the PE stream stays dense across groups."""
                qk_sb = qk_pool.tile([128, 4, GP], F32R, tag="qk",
                                     name=f"qk{g}")
                v_aug = v_pool.tile([128, MC_G, H, 64], BF16, tag="va",
                                    name=f"va{g}")
                units = []

                def a1_unit(rr):
                    def _go():
                        # two 1-bank tiles so the misc pool stays 1-bank
                        pss = [ps_misc.tile([128, 512], F32, tag="ms",
                                            name=f"psa{g}_{rr}_{n2}")
                               for n2 in range(2)]
                        for n2 in range(2):
                            for k in range(2):
                                nc.tensor.matmul(
                                    pss[n2],
                                    wqk_sb[:, k, rr * 128:(rr + 1) * 128],
                                    xg[:, k, n2 * 512:(n2 + 1) * 512],
                                    start=(k == 0), stop=(k == 1),
                                )
                        for n2 in range(2):
                            if rr < 4 - A1_SCALAR:
                                nc.vector.tensor_scalar_add(
                                    qk_sb[:, rr, n2 * 512:(n2 + 1) * 512],
                                    pss[n2], bqk_sb[:, rr:rr + 1],
                                )
                            else:
                                # scalar-engine bias add (Identity shares
                                # the ln/exp act table set: no reload)
                                nc.scalar.activation(
                                    qk_sb[:, rr, n2 * 512:(n2 + 1) * 512],
                                    pss[n2], IDENT,
                                    bias=bqk_sb[:, rr:rr + 1],
                                )
                    return _go

                def mask_unit(rr):
                    def _go():
                        nc.vector.tensor_tensor(
                            qk_sb[:, rr, :], qk_sb[:, rr, :], mk_sb, MUL
                        )
                    return _go

                def memset_unit():
                    def _go():
                        if TAIL_DIRECT and g == NG - 1:
                            # last group: full ones block for the direct
                            # tail reciprocal (32 rowsum copies)
                            nc.gpsimd.memset(v_aug[:, :, :, 32:64], 1.0)
                        else:
                            nc.gpsimd.memset(v_aug[:, :, :, 32:33], 1.0)
                    return _go

                def a2_unit(mc0):
                    def _go():
                        ps = ps_misc.tile([128, 2, 256], F32, tag="ms",
                                          name=f"psv{g}_{mc0}")
                        for d in range(2):
                            mc = mc0 + d
                            for k in range(2):
                                nc.tensor.matmul(
                                    ps[:, d, :],
                                    xg[:, k, mc * 128:(mc + 1) * 128],
                                    wv_sb[:, k, :],
                                    start=(k == 0), stop=(k == 1),
                                )
                        for d in range(2):
                            mc = mc0 + d
                            if masked:
                                nc.vector.tensor_scalar_mul(
                                    v_aug[:, mc, :, 0:32],
                                    ps[:, d, :].rearrange(
                                        "p (h d2) -> p h d2", h=H),
                                    mkp_sb[:, g * MC_G + mc:
                                           g * MC_G + mc + 1],
                                )
                            else:
                                nc.vector.tensor_copy(
                                    v_aug[:, mc, :, 0:32],
                                    ps[:, d, :].rearrange(
                                        "p (h d2) -> p h d2", h=H),
                                )
                    return _go

                units.append(memset_unit())
                for rr in range(4):
                    units.append(a1_unit(rr))
                if masked:
                    units.append(mask_unit(2))
                    units.append(mask_unit(3))
                for mc0 in range(0, MC_G, 2):
                    units.append(a2_unit(mc0))
                return qk_sb, v_aug, units

            def emit_st_exp(g, qk_sb, t, hg, j_global):
                # ST = K@Q^T then exp, two 2-head sub-batches
                tsl = slice(t * 256, (t + 1) * 256)
                et4 = et_pool.tile([128, 4, 512], BF16, tag="et",
                                   name=f"et{j_global}")
                st2s = [ps_st.tile([128, 2, 512], F32, tag="st",
                                   name=f"st{j_global}_{sb}")
                        for sb in range(2)]
                # tile-major order (banks alternate within each tile): the
                # first exp's tile is fully written after 4 matmuls, not 6
                if ST_ORDER == "tile":
                    order = [(j, hh) for hh2 in (0, 2)
                             for j in range(2) for hh in (hh2, hh2 + 1)]
                else:
                    order = [(j, hh) for j in range(2) for hh in range(4)]
                for j, hh in order:
                    po = hh * 32
                    nc.tensor.matmul(
                        st2s[hh // 2][:, hh % 2, j * 256:(j + 1) * 256],
                        qk_sb[po:po + 32, 2 + hg,
                              t * 256 + j * 128:t * 256 + (j + 1) * 128],
                        qk_sb[po:po + 32, hg, tsl],
                        start=True, stop=True,
                        tile_position=(po, 0),
                    )
                for sb in range(2):
                    nc.scalar.activation(
                        et4[:, sb * 2:(sb + 1) * 2, :], st2s[sb], EXP
                    )
                return et4

            def emit_ctx(g, v_aug, t, hg, et4, j_global):
                # [ctx^T ; rowsum] accumulate: lhsT = [V_h | 1]
                direct = TAIL_DIRECT and j_global >= NIT - 2
                if direct:
                    # 33 rowsum copies; last iter's tile borrows from the
                    # draining ST pool so ps_acc never serializes the tail
                    pool = ps_st if j_global == NIT - 1 else ps_acc
                    tag = "st" if j_global == NIT - 1 else "c4"
                    acc4 = pool.tile([64, 4, 256], F32, tag=tag,
                                     name=f"acc{j_global}")
                else:
                    acc4 = ps_acc.tile([33, 4, 256], F32, tag="c4",
                                       name=f"acc{j_global}")
                ncol = 64 if direct else 33
                for hh in range(4):
                    h = hg * 4 + hh
                    for j in range(2):
                        nc.tensor.matmul(
                            acc4[:, hh, :],
                            v_aug[:, t * 2 + j, h, 0:ncol],
                            et4[:, hh, j * 256:(j + 1) * 256],
                            start=(j == 0), stop=(j == 1),
                        )
                return acc4

            def emit_direct_norm(j, acc4, g, t, hg):
                """Tail path: classic in-place reciprocal (scalar is idle
                once the score exps end) -- no DMA round trip."""
                tsl = slice(t * 256, (t + 1) * 256)
                ln4 = z_pool.tile([32, 4, 256], F32, tag="dln",
                                  name=f"dln{j}", bufs=2)
                rcp4 = z_pool.tile([32, 4, 256], F32, tag="drc",
                                   name=f"drc{j}", bufs=2)
                nc.scalar.activation(ln4, acc4[32:64, :, :], LOG)
                nc.scalar.activation(rcp4, ln4, EXP, scale=-1.0)
                ctxT_sb = ctxT_tiles[g]
                for hh in range(4):
                    nc.vector.tensor_tensor(
                        ctxT_sb[hh * 32:(hh + 1) * 32, hg, tsl],
                        acc4[0:32, hh, :], rcp4[:, hh, :], MUL,
                    )

            # ---- reciprocal dance stages (batched per iteration-pair) -----
            state = {}   # j_global -> dict with tiles + (g, t, hg)
            pstate = {}  # pair index p -> dict with zrow/rcp4 tiles

            def emit_copy_t1(j):
                st = state[j]
                p, half = j // 2, j % 2
                if half == 0:
                    cacc = cacc_pool.tile([33, 2, 4, 256], BF16, tag="cacc",
                                          name=f"cacc{p}")
                    pstate[p] = dict(cacc=cacc)
                cacc = pstate[p]["cacc"]
                nc.vector.tensor_copy(cacc[:, half, :, :], st["acc4"])
                st["cacc"] = cacc
                st["half"] = half
                st["acc4"] = None
                if half == 1:
                    zrow = z_pool.tile([128, 16], BF16, tag="zrow",
                                       name=f"zrow{p}")
                    # SP queue holds only T1/T3 dance triggers, so its
                    # blocking waits never stall a compute engine's stream
                    nc.sync.dma_start(out=zrow, in_=cacc[32:33, :, :, :])
                    pstate[p]["zrow"] = zrow

            def emit_recip_t2(p):
                ps = pstate[p]
                zl = z_pool.tile([128, 16], F32, tag="zl", name=f"zl{p}")
                zrcp = z_pool.tile([128, 16], BF16, tag="zrcp",
                                   name=f"zrcp{p}")
                nc.scalar.activation(zl, ps["zrow"], LOG)
                nc.scalar.activation(zrcp, zl, EXP, scale=-1.0)
                ps["zrow"] = None
                # trigger from the Act queue right after the exp: no wait
                nc.scalar.dma_start(out=rdz[p, :], in_=zrcp)
                ps["zrcp"] = zrcp

            def emit_t3(p):
                ps = pstate[p]
                rcp4 = rcp4_pool.tile([32, 2, 4, 256], BF16, tag="rcp4",
                                      name=f"rcp4_{p}")
                nc.sync.dma_start(out=rcp4,
                                  in_=rdz[p, :].partition_broadcast(32))
                ps["rcp4"] = rcp4

            def emit_mult(j, ctxT_tiles):
                st = state.pop(j)
                p, half = j // 2, j % 2
                g, t, hg = st["g"], st["t"], st["hg"]
                tsl = slice(t * 256, (t + 1) * 256)
                ctxT_sb = ctxT_tiles[g]
                rcp4 = pstate[p]["rcp4"]
                for hh in range(4):
                    # drain tail: split the serial Pool mult backlog onto
                    # the then-idle DVE (bf16 2x rate)
                    on_dve = (hh < MULT_DVE or
                              (j >= NIT - TAIL_DVE and hh % 2 == 0))
                    eng = nc.vector if on_dve else nc.gpsimd
                    if MULT_OP == "stt" and eng is nc.gpsimd:
                        eng.scalar_tensor_tensor(
                            ctxT_sb[hh * 32:(hh + 1) * 32, hg, tsl],
                            st["cacc"][0:32, half, hh, :], 1.0,
                            rcp4[:, half, hh, :],
                            mybir.AluOpType.bypass, MUL,
                        )
                    else:
                        eng.tensor_tensor(
                            ctxT_sb[hh * 32:(hh + 1) * 32, hg, tsl],
                            st["cacc"][0:32, half, hh, :],
                            rcp4[:, half, hh, :], MUL,
                        )
                if half == 1:
                    pstate.pop(p)

            def dance_pre(k, ctxT_tiles):
                """Stages for older iterations, run at iteration-slot k.
                Pair p's chain: T1 fires with copy(2p+1) at slot
                2p+1+CTX_LAG, recip+T2 one slot later, T3 the next, mults
                at 2p+MULT_LAG / 2p+1+MULT_LAG."""
                if k - MULT_LAG in state:
                    emit_mult(k - MULT_LAG, ctxT_tiles)
                if T3_SLOT > 0 and (k - 2 - CTX_LAG - T3_SLOT) >= 0 \
                        and (k - 2 - CTX_LAG - T3_SLOT) % 2 == 0:
                    p = (k - 2 - CTX_LAG - T3_SLOT) // 2
                    if p in pstate and "rcp4" not in pstate[p]:
                        emit_t3(p)
                if (k - 2 - CTX_LAG) >= 0 and (k - 2 - CTX_LAG) % 2 == 0:
                    p = (k - 2 - CTX_LAG) // 2
                    if p in pstate and "zrcp" not in pstate[p]:
                        emit_recip_t2(p)
                        if T3_SLOT == 0 or p >= NIT // 2 - TAIL_T3:
                            # tail pairs: T3 in the same slot (the SP queue
                            # is quiet there, and the shorter chain lets the
                            # final mults start a slot earlier)
                            emit_t3(p)

            def emit_c_units(g, ctxT_sb):
                o_sb = o_pool.tile([128, MC_G, 256], F32, tag="o",
                                   name=f"o{g}")

                def c_unit(mc0):
                    def _go():
                        ps = ps_misc.tile([128, 2, 256], F32, tag="ms",
                                          name=f"psc{g}_{mc0}")
                        for d in range(2):
                            mc = mc0 + d
                            for kc in range(2):
                                nc.tensor.matmul(
                                    ps[:, d, :],
                                    ctxT_sb[:, kc, mc * 128:(mc + 1) * 128],
                                    wl_sb[:, kc, :],
                                    start=(kc == 0), stop=(kc == 1),
                                )
                        for d in range(2):
                            mc = mc0 + d
                            nc.vector.tensor_tensor(
                                o_sb[:, mc, :], ps[:, d, :], bfin_sb, ADD
                            )
                        # ship this 256-point slice immediately: overlaps
                        # the store with remaining compute, shrinking the tail
                        rsl = slice(g * GP + mc0 * 128,
                                    g * GP + (mc0 + 2) * 128)
                        nc.sync.dma_start(
                            out=out[rsl, :].rearrange(
                                "(m p) n -> p m n", p=128),
                            in_=o_sb[:, mc0:mc0 + 2, :],
                        )
                    return _go

                return [c_unit(mc0) for mc0 in range(0, MC_G, 2)]

            def weave(c_units, a_units):
                """Readiness-greedy interleave.  c_unit[ci] becomes ready
                when the norm mults of track ci of the previous group have
                executed (slot 2*ci+MULT_LAG-8 of this group, + margin);
                a-units wait on the xg DMA issued at group start (A_DELAY
                slots), except the dependency-free memset."""
                if os.environ.get("WEAVE", "thr") == "greedy":
                    cand = []
                    for ci, u in enumerate(c_units):
                        rdy = max(0.0,
                                  2 * ci + MULT_LAG - 8 + 0.5 * C_MARGIN)
                        cand.append([rdy, 1, ci, u])
                    for ai, u in enumerate(a_units):
                        rdy = 0.0 if ai == 0 else A_DELAY
                        cand.append([rdy, 0, ai, u])
                    total = len(cand)
                    units = []
                    remaining = sorted(cand)
                    for s in range(total):
                        slot = 8.0 * s / total
                        pick = None
                        for ent in remaining:
                            if ent[0] <= slot:
                                pick = ent
                                break
                        if pick is None:
                            pick = remaining[0]
                        remaining.remove(pick)
                        units.append(pick[3])
                    return units
                total = len(c_units) + len(a_units)
                units = []
                ai = ci = 0
                for s in range(total):
                    thr = None
                    if ci < len(c_units):
                        thr = total * (2 * ci + MULT_LAG - 7
                                       + C_MARGIN) / 8.0
                    if thr is not None and (s >= thr or ai >= len(a_units)):
                        units.append(c_units[ci])
                        ci += 1
                    else:
                        units.append(a_units[ai])
                        ai += 1
                return units

            # ---- software-pipelined schedule ------------------------------
            xgs = {0: emit_load(0)}
            load_late_consts()
            qk0, va0, units0 = emit_a_units(0, *xgs.pop(0))
            for u in units0:
                u()
            if XG_EARLY:
                xgs[1] = emit_load(1)
            ab = {0: (qk0, va0)}
            ab_v = {}
            ctx_q = []
            ctxT_tiles = {}
            c_carry = []
            iters = [(t, hg) for t in range(TPG) for hg in (0, 1)]
            for g in range(NG):
                qk_sb, v_aug = ab.pop(g)
                ctxT_tiles[g] = ctx_pool.tile([128, 2, GP], BF16, tag="ctxT",
                                              name=f"ctxT{g}")
                a_units = []
                if g + 1 < NG:
                    if g + 1 not in xgs:
                        xgs[g + 1] = emit_load(g + 1)
                    qkn, van, aun = emit_a_units(g + 1, *xgs.pop(g + 1))
                    ab[g + 1] = (qkn, van)
                    a_units = aun
                units = weave(list(c_carry), a_units)
                c_carry = []
                ui = 0

                def pop_ctx(gq, entry):
                    pk, pt, phg, pet = entry
                    acc4 = emit_ctx(gq, ab_v[pk // 8], pt, phg, pet, pk)
                    if TAIL_DIRECT and pk >= NIT - 2:
                        emit_direct_norm(pk, acc4, gq, pt, phg)
                    else:
                        state[pk] = dict(acc4=acc4, g=gq, t=pt, hg=phg)
                        emit_copy_t1(pk)

                ab_v[g] = v_aug
                for i, (t, hg) in enumerate(iters):
                    k = g * 8 + i
                    dance_pre(k, ctxT_tiles)
                    if XG_EARLY and g + 2 < NG and 1 <= i <= 4:
                        # issue the g+2 input load a full group before its
                        # first consumer, in four slot-staggered chunks so
                        # the burst never crowds out the dance DMAs
                        if i == 1:
                            xgs[g + 2] = alloc_xg(g + 2)
                        load_xg_chunk(xgs[g + 2], g + 2, i - 1, 4)
                    take = (len(units) * (i + 1)) // len(iters) - ui
                    # half the units go before ST: PE filler while the ST
                    # tiles wait on the previous exp to release PSUM (but
                    # not at slot 0: there ST is the ready work)
                    pre = take // 2 if (i > 0 or PRE0) else 0
                    for _ in range(pre):
                        units[ui]()
                        ui += 1
                        take -= 1
                    et4 = emit_st_exp(g, qk_sb, t, hg, k)
                    while ctx_q and ctx_q[0][0] <= k - CTX_LAG:
                        pop_ctx(ctx_q[0][0] // 8, ctx_q.pop(0))
                    ctx_q.append((k, t, hg, et4))
                    for _ in range(take):
                        units[ui]()
                        ui += 1
                while ui < len(units):
                    units[ui]()
                    ui += 1
                # group-final: drain pending ctx before the boundary (the
                # ST pipeline restarts cleanly in the next group anyway)
                while ctx_q:
                    pop_ctx(ctx_q[0][0] // 8, ctx_q.pop(0))
                c_carry = emit_c_units(g, ctxT_tiles[g])

            # ---- epilogue: drain the dance, last group's C phase ----------
            k = NG * 8
            while state:
                dance_pre(k, ctxT_tiles)
                k += 1
            for u in c_carry:
                u()

    return nc


_PROG_CACHE = {}


def _get_program(masked: bool):
    if masked not in _PROG_CACHE:
        _PROG_CACHE[masked] = _build_program(masked)
    return _PROG_CACHE[masked]


def _prep_host(values, w_qkv, b_qkv, w_lin, b_lin):
    """Host-side weight preprocessing (all cheap, shared across cores)."""
    scale = 1.0 / np.sqrt(DH)
    w_qkv = np.asarray(w_qkv, np.float32).copy()
    b_qkv = np.asarray(b_qkv, np.float32).copy()
    w_lin = np.asarray(w_lin, np.float32)
    b_lin = np.asarray(b_lin, np.float32)
    w_qkv[:DOUT] *= scale
    b_qkv[:DOUT] *= scale

    wqk = np.ascontiguousarray(
        w_qkv[:2 * DOUT].T.reshape(2, 128, 512)
    )  # [k-chunk, k-part, row]
    wv = np.ascontiguousarray(w_qkv[2 * DOUT:].T.reshape(2, 128, 256))
    wl = np.ascontiguousarray(
        w_lin.T.reshape(2, 128, 256)).astype(ml_dtypes.bfloat16)
    bqk = np.ascontiguousarray(b_qkv[:2 * DOUT].reshape(4, 128).T)
    b_v = b_qkv[2 * DOUT:]  # unscaled: only the q section was scaled above
    bfin = (w_lin @ b_v + b_lin).astype(np.float32)
    return wqk, wv, wl, bqk, bfin


def _run(values_padded, mask, w_arrs, trace=False):
    """values_padded: [N, 256] in track-padded order; mask: None or [N]."""
    wqk, wv, wl, bqk, bfin = w_arrs
    masked = mask is not None
    nc = _get_program(masked)

    in_maps = []
    for c in range(N_CORES):
        sl = slice(c * PC, (c + 1) * PC)
        xTc = np.ascontiguousarray(
            values_padded[sl].T.reshape(2, 128, PC)
        )
        m = dict(xT=xTc, wqk=wqk, wv=wv, wl=wl, bqk=bqk, bfin=bfin)
        if masked:
            mc_ = np.ascontiguousarray(mask[sl], np.float32)
            m["mkf"] = mc_
            m["mkp"] = np.ascontiguousarray(mc_.reshape(PC // 128, 128).T)
        in_maps.append(m)

    if trace:
        _ensure_ntff_hook()
    res = run_bass_kernel_spmd(nc, in_maps, list(range(N_CORES)), trace=trace)
    outp = np.concatenate([res.results[c]["out"] for c in range(N_CORES)], 0)
    return outp, res


LAST_RESULTS = None


def kernel(values, w_qkv, b_qkv, w_lin, b_lin, track_ids, n_tracks,
           num_heads, _trace=False):
    global LAST_RESULTS
    values = np.asarray(values, np.float32)
    track_ids = np.asarray(track_ids, np.int32)
    n_tracks_i = int(n_tracks)
    num_heads_i = int(num_heads)
    assert values.shape == (N, DIN) and n_tracks_i == T and num_heads_i == H, (
        "kernel compiled for N=65536, d=256, T=256, H=8"
    )

    w_arrs = _prep_host(values, w_qkv, b_qkv, w_lin, b_lin)

    counts = np.bincount(track_ids, minlength=T)
    equal = bool((counts == L).all())

    if equal:
        outp, res = _run(values, None, w_arrs, trace=_trace)
        LAST_RESULTS = res
        return outp

    # general sorted-ragged path: scatter to padded [T, L] grid on host,
    # run the same device kernel with padding masked out of K and V, then
    # gather back (mirroring jax's oob-drop scatter / clip gather).
    starts = np.concatenate([[0], np.cumsum(counts)[:-1]])
    pos = np.arange(N, dtype=np.int64) - starts[track_ids]
    keep = pos < L
    rows = track_ids.astype(np.int64) * L + np.minimum(pos, L - 1)
    padded = np.zeros((T * L, DIN), np.float32)
    padded[rows[keep]] = values[keep]
    mask = np.zeros(T * L, np.float32)
    mask[rows[keep]] = 1.0
    outp, res = _run(padded, mask, w_arrs, trace=_trace)
    LAST_RESULTS = res
    return np.ascontiguousarray(outp[rows])


# revision 69
# speedup vs baseline: 1.0158x; 1.0158x over previous
"""Trainium2 Bass kernel for a track-wise (ragged-sequence) attention layer.

Math (per track t of length L, per head h):
    qkv = values @ w_qkv.T + b_qkv                      # [N, 3*256]
    S   = q k^T / sqrt(Dh);  P = softmax(S);  ctx = P v
    out = ctx @ w_lin.T + b_lin

Device strategy (data parallel over tracks, 8 cores, no cross-core comm):
  - scores are computed directly transposed per head: ST = K @ Q^T, so
    exp(ST) is exactly the lhs^T the ctx matmul needs -- no PE transposes.
  - no max-subtraction in softmax (scores are ~N(0,1); exp is safe in fp32).
  - the softmax denominator is folded into the ctx matmul: lhsT = [V_h | 1],
    so one PE pass yields [ctx^T; rowsum] in PSUM (a single ones column;
    32 ones columns only for the last two iterations, see tail below).
  - RECIPROCAL DANCE: scalar-engine activation cost is free-size-per-lane,
    so the old [32,4,256]-layout ln/exp reciprocal cost 2.2us/iter for 1024
    unique values.  Now: one DVE copy drains [ctx^T; rowsum] PSUM->SBUF
    (bf16), per iteration-PAIR one SP-queue DMA reshapes the 2048 rowsums
    [1,2048]->[128,16], scalar ln+exp runs at 16 elem/lane (~0.5us/pair),
    an Act-queue DMA writes a DRAM slot, and an SP-queue DMA broadcasts
    back to [32,2,4,256].  Scalar recip cost: 139us -> ~15us total.
  - the normalize multiply runs on the otherwise-idle GPSIMD/Pool engine
    (SBUF x SBUF); the last 4 iterations split it Pool/DVE, and the last 2
    use a direct path (32 ones-columns + in-place scalar ln/exp + DVE mult
    from PSUM) because the scalar engine is idle once score exps end.
  - bf16 for et4/v_aug/ctx^T/cacc/rcp4/w_lin: same 1 cyc/row PE rate,
    halves SBUF, funds the lag-5 software pipeline (norm mult of iter j
    lands at slot j+5).
  - PSUM pooling: ST tiles (4 banks, depth 2), the ctx/rowsum accumulator
    (2 banks, single-buffered -- the DVE copy drains it within a slot), and
    1-bank tiles for the a1/a2/c projections (2 banks).  Separating the
    projection tiles from the ST pool stops the PE stalling on exp's PSUM
    reads (that stall was resetting the PE DVFS state every iteration:
    mid-clock 1.2GHz costs 2x cycles until 3us of continuous busy).
  - ST matmuls issue tile-major with alternating banks so the first exp's
    tile completes after 4 matmuls, not 6 (never >1 consecutive write to
    the same PSUM bank -- consecutive same-bank writes serialize).
  - DMA queue discipline: xg loads + dance T1/T3 + output stores on the SP
    queue, dance T2 on the Act queue right after the exp that feeds it
    (zero blocking wait), consts/memset on Pool.  Queue assignment matters
    more than engine busy: a blocked DMA trigger stalls its whole in-order
    queue (that serialization cost ~60us in earlier revisions).
  - weave: the next group's projections and the previous group's output
    units interleave through the attention iterations; c_unit[ci] is placed
    past slot (2*ci + MULT_LAG - 7 + C_MARGIN) where its norm mults have
    executed.
  - 1/sqrt(Dh) is folded into w_q/b_q on the host; b_v is folded into the
    final bias (softmax rows sum to 1): b_final = w_lin @ b_v + b_lin.
  - fp32 matmuls run as float32r (full rate with >=256 free out).

History: 812us (first session) -> 354us (second session, scalar-bound:
276us scalar of which 139us was the reciprocal) -> 242-247us (this
session).  Engine busy now ~PE 210 (DVFS-mixed), Pool 170, DVE 170,
scalar 155us.  Measured dead ends this session: custom-DVE ops and
gpsimd.partition_broadcast ("ISA wrong length" in this walrus build),
ldw-opt with 33-col bf16 lhsT, CTX_LAG=2 exp decoupling, readiness-greedy
weave, xg loads issued a group early (DMA collisions with the dance),
C_MARGIN=2 (327us!), fp8 needs DoubleRow layout gymnastics to beat bf16.
"""

import os
import sys

import numpy as np

for _p in ("/opt/trn_rl_repo", "/root/.axon_site/_ro/trn_rl_repo"):
    if os.path.isdir(_p) and _p not in sys.path:
        sys.path.append(_p)

import ml_dtypes

import concourse.bass as bass
import concourse.tile as tile
from concourse import mybir
from concourse.bass_utils import run_bass_kernel_spmd

F32 = mybir.dt.float32
F32R = mybir.dt.float32r
BF16 = mybir.dt.bfloat16
EXP = mybir.ActivationFunctionType.Exp
LOG = mybir.ActivationFunctionType.Ln
IDENT = mybir.ActivationFunctionType.Identity
MUL = mybir.AluOpType.mult
ADD = mybir.AluOpType.add

N_CORES = 8
N, DIN, DOUT, H, T, L = 65536, 256, 256, 8, 256, 256

DH = DOUT // H          # 32
PC = N // N_CORES       # 8192 points per core
TPC = T // N_CORES      # 32 tracks per core
TPG = 4                 # tracks per group
NG = TPC // TPG         # 8 groups
GP = TPG * L            # 1024 points per group
MC_G = GP // 128        # 8 128-point chunks per group
NIT = NG * TPG * 2      # 64 iterations total

# tuning knobs
MULT_LAG = int(os.environ.get("MULT_LAG", "5"))
CTX_LAG = int(os.environ.get("CTX_LAG", "1"))   # slots between ST and ctx
T3_SLOT = int(os.environ.get("T3_SLOT", "1"))   # T3 delay after T2 (slots)
ST_ORDER = os.environ.get("ST_ORDER", "tile")   # j | tile matmul order
TAIL_T3 = int(os.environ.get("TAIL_T3", "0"))   # last pairs: same-slot T3
C_MARGIN = int(os.environ.get("C_MARGIN", "3"))
XG_EARLY = os.environ.get("XG_EARLY", "0") == "1"
PRE0 = os.environ.get("PRE0", "1") == "1"
A_DELAY = float(os.environ.get("A_DELAY", "1.5"))  # xg DMA latency, slots
XG_Q = os.environ.get("XG_Q", "sync")   # queue for xg load triggers
MULT_DVE = int(os.environ.get("MULT_DVE", "0"))  # of 4 norm-mults on DVE
MULT_OP = os.environ.get("MULT_OP", "tt")        # tt | stt pool ucode
TAIL_DVE = int(os.environ.get("TAIL_DVE", "4"))  # last iters: mults on DVE
TAIL_DIRECT = os.environ.get("TAIL_DIRECT", "1") == "1"  # last 2 iters:
# 32 ones-columns + direct scalar ln/exp + DVE mult (no DMA dance)
A1_SCALAR = int(os.environ.get("A1_SCALAR", "0"))   # of 4 rr units on scalar
DANCE_SYNC = os.environ.get("DANCE_SYNC", "1") == "1"  # dance DMAs on SP


class _TileContext(tile.TileContext):
    """TileContext whose final drain carries at most one semaphore wait per
    instruction (the walrus build in this container rejects multi-wait
    TPB_CTRL instructions)."""

    def _drain_and_barrier(self, tick_clock, wait_clock):
        super()._drain_and_barrier(tick_clock, wait_clock)
        self._split_multi_waits()

    def _split_multi_waits(self):
        nc = self.nc
        for f in nc.m.functions:
            for bb in f.blocks:
                changed = False
                new_insts = []
                for inst in bb.instructions:
                    si = inst.sync_info
                    if si is not None and len(si.on_wait) > 1:
                        waits = list(si.on_wait)
                        for w in waits[:-1]:
                            nop = mybir.InstNoOp(
                                name=f"I-{nc.next_id()}", ins=[], outs=[]
                            )
                            nop.engine = inst.engine
                            nop.sync_info = mybir.SyncInfo(
                                on_wait=[w], on_update=[]
                            )
                            new_insts.append(nop)
                        inst.sync_info = mybir.SyncInfo(
                            on_wait=[waits[-1]], on_update=list(si.on_update)
                        )
                        changed = True
                    new_insts.append(inst)
                if changed:
                    bb.instructions = new_insts


def _ensure_ntff_hook():
    """The agent image's ``antenv`` lacks ``axon_hooks``; provide it so
    ``run_bass_kernel_spmd(trace=True)`` can register the axon NTFF hook."""
    try:
        import antenv.axon_hooks  # noqa: F401
        return
    except ImportError:
        pass
    import types

    import antenv

    mod = types.ModuleType("antenv.axon_hooks")
    _hook = [None]
    mod.set_axon_ntff_profile_hook = lambda h: _hook.__setitem__(0, h)
    mod.get_axon_ntff_profile_hook = lambda: _hook[0]
    sys.modules["antenv.axon_hooks"] = mod
    antenv.axon_hooks = mod
    try:
        from trn_agent_boot.trn_boot import _ntff_profile_via_ctypes

        mod.set_axon_ntff_profile_hook(
            _ntff_profile_via_ctypes("/opt/axon/libaxon_pjrt.so")
        )
    except Exception as e:  # pragma: no cover - tracing is best-effort
        print(f"ntff hook setup failed: {e}", file=sys.stderr)


def _build_program(masked: bool):
    nc = bass.Bass("TRN2", target_bir_lowering=False, debug=False,
                   num_devices=N_CORES)

    xT = nc.dram_tensor("xT", [2, 128, PC], F32R, kind="ExternalInput").ap()
    wqk = nc.dram_tensor("wqk", [2, 128, 512], F32R, kind="ExternalInput").ap()
    wv = nc.dram_tensor("wv", [2, 128, 256], F32R, kind="ExternalInput").ap()
    wl = nc.dram_tensor("wl", [2, 128, 256], BF16, kind="ExternalInput").ap()
    bqk = nc.dram_tensor("bqk", [128, 4], F32, kind="ExternalInput").ap()
    bfin = nc.dram_tensor("bfin", [256], F32, kind="ExternalInput").ap()
    rdz = nc.dram_tensor("rdz", [NIT // 2, 2048], BF16, kind="Internal").ap()
    if masked:
        mkf = nc.dram_tensor("mkf", [PC], F32, kind="ExternalInput").ap()
        mkp = nc.dram_tensor("mkp", [128, PC // 128], F32,
                             kind="ExternalInput").ap()
    out = nc.dram_tensor("out", [PC, DOUT], F32, kind="ExternalOutput").ap()

    dq = nc.sync if DANCE_SYNC else nc.gpsimd

    with _TileContext(nc) as tc:
        with (
            tc.tile_pool(name="consts", bufs=1) as consts,
            tc.tile_pool(name="xg", bufs=3) as xg_pool,
            tc.tile_pool(name="qk", bufs=2) as qk_pool,
            tc.tile_pool(name="vsb", bufs=3) as v_pool,
            tc.tile_pool(name="et", bufs=5) as et_pool,
            tc.tile_pool(name="ctxT", bufs=3) as ctx_pool,
            tc.tile_pool(name="osb", bufs=2) as o_pool,
            tc.tile_pool(name="cacc", bufs=(MULT_LAG + 5) // 2) as cacc_pool,
            tc.tile_pool(name="zd", bufs=3) as z_pool,
            tc.tile_pool(name="rcp4", bufs=4) as rcp4_pool,
            tc.tile_pool(name="ps_st", bufs=2, space="PSUM") as ps_st,
            tc.tile_pool(name="ps_acc", bufs=1, space="PSUM") as ps_acc,
            tc.tile_pool(name="ps_misc", bufs=2, space="PSUM") as ps_misc,
        ):
            # ---- constants -------------------------------------------------
            # only wqk/bqk gate the first projection; defer the other
            # const loads until after group 0's input is on the queue
            wqk_sb = consts.tile([128, 2, 512], F32R)
            nc.gpsimd.dma_start(out=wqk_sb, in_=wqk.rearrange("k p r -> p k r"))
            bqk_sb = consts.tile([128, 4], F32)
            nc.gpsimd.dma_start(out=bqk_sb, in_=bqk)
            wv_sb = consts.tile([128, 2, 256], F32R)
            wl_sb = consts.tile([128, 2, 256], BF16)
            bfin_sb = consts.tile([128, 256], F32)

            def load_late_consts():
                nc.gpsimd.dma_start(out=wv_sb,
                                    in_=wv.rearrange("k p r -> p k r"))
                nc.gpsimd.dma_start(out=wl_sb,
                                    in_=wl.rearrange("k p r -> p k r"))
                nc.gpsimd.dma_start(out=bfin_sb,
                                    in_=bfin.partition_broadcast(128))
            if masked:
                mkp_sb = consts.tile([128, PC // 128], F32)
                nc.gpsimd.dma_start(out=mkp_sb, in_=mkp)

            def alloc_xg(g):
                xg = xg_pool.tile([128, 2, GP], F32R, tag="xg", name=f"xg{g}")
                mk_sb = None
                if masked:
                    gsl = slice(g * GP, (g + 1) * GP)
                    mk_sb = et_pool.tile([128, GP], F32, tag="mk",
                                         name=f"mk{g}", bufs=2)
                    nc.gpsimd.dma_start(
                        out=mk_sb, in_=mkf[gsl].partition_broadcast(128)
                    )
                return xg, mk_sb

            def load_xg_chunk(xgmk, g, ci, nch):
                """One 1/nch slice of the group input (no upstream deps:
                DRAM src, long-free tile -- never blocks its queue)."""
                xg, _ = xgmk
                w = GP // nch
                hsl = slice(g * GP + ci * w, g * GP + (ci + 1) * w)
                eng = {"sync": nc.sync, "scalar": nc.scalar,
                       "gpsimd": nc.gpsimd}[XG_Q]
                eng.dma_start(
                    out=xg[:, :, ci * w:(ci + 1) * w],
                    in_=xT[:, :, hsl].rearrange("k p n -> p k n"),
                )

            def emit_load(g):
                xgmk = alloc_xg(g)
                for hf in range(2):
                    load_xg_chunk(xgmk, g, hf, 2)
                return xgmk

            def emit_a_units(g, xg, mk_sb):
                """Return (qk_sb, v_aug, units): units are small emission
                thunks (2 MMs + 1 DVE/scalar op each) to weave between B
                iterations so the PE stream stays dense across groups."""
                qk_sb = qk_pool.tile([128, 4, GP], F32R, tag="qk",
                                     name=f"qk{g}")
                v_aug = v_pool.tile([128, MC_G, H, 64], BF16, tag="va",
                                    name=f"va{g}")
                units = []

                def a1_unit(rr):
                    def _go():
                        # two 1-bank tiles so the misc pool stays 1-bank
                        pss = [ps_misc.tile([128, 512], F32, tag="ms",
                                            name=f"psa{g}_{rr}_{n2}")
                               for n2 in range(2)]
                        for n2 in range(2):
                            for k in range(2):
                                nc.tensor.matmul(
                                    pss[n2],
                                    wqk_sb[:, k, rr * 128:(rr + 1) * 128],
                                    xg[:, k, n2 * 512:(n2 + 1) * 512],
                                    start=(k == 0), stop=(k == 1),
                                )
                        for n2 in range(2):
                            if rr < 4 - A1_SCALAR:
                                nc.vector.tensor_scalar_add(
                                    qk_sb[:, rr, n2 * 512:(n2 + 1) * 512],
                                    pss[n2], bqk_sb[:, rr:rr + 1],
                                )
                            else:
                                # scalar-engine bias add (Identity shares
                                # the ln/exp act table set: no reload)
                                nc.scalar.activation(
                                    qk_sb[:, rr, n2 * 512:(n2 + 1) * 512],
                                    pss[n2], IDENT,
                                    bias=bqk_sb[:, rr:rr + 1],
                                )
                    return _go

                def mask_unit(rr):
                    def _go():
                        nc.vector.tensor_tensor(
                            qk_sb[:, rr, :], qk_sb[:, rr, :], mk_sb, MUL
                        )
                    return _go

                def memset_unit():
                    def _go():
                        if TAIL_DIRECT and g == NG - 1:
                            # last group: full ones block for the direct
                            # tail reciprocal (32 rowsum copies)
                            nc.gpsimd.memset(v_aug[:, :, :, 32:64], 1.0)
                        else:
                            nc.gpsimd.memset(v_aug[:, :, :, 32:33], 1.0)
                    return _go

                def a2_unit(mc0):
                    def _go():
                        ps = ps_misc.tile([128, 2, 256], F32, tag="ms",
                                          name=f"psv{g}_{mc0}")
                        for d in range(2):
                            mc = mc0 + d
                            for k in range(2):
                                nc.tensor.matmul(
                                    ps[:, d, :],
                                    xg[:, k, mc * 128:(mc + 1) * 128],
                                    wv_sb[:, k, :],
                                    start=(k == 0), stop=(k == 1),
                                )
                        for d in range(2):
                            mc = mc0 + d
                            if masked:
                                nc.vector.tensor_scalar_mul(
                                    v_aug[:, mc, :, 0:32],
                                    ps[:, d, :].rearrange(
                                        "p (h d2) -> p h d2", h=H),
                                    mkp_sb[:, g * MC_G + mc:
                                           g * MC_G + mc + 1],
                                )
                            else:
                                nc.vector.tensor_copy(
                                    v_aug[:, mc, :, 0:32],
                                    ps[:, d, :].rearrange(
                                        "p (h d2) -> p h d2", h=H),
                                )
                    return _go

                units.append(memset_unit())
                for rr in range(4):
                    units.append(a1_unit(rr))
                if masked:
                    units.append(mask_unit(2))
                    units.append(mask_unit(3))
                for mc0 in range(0, MC_G, 2):
                    units.append(a2_unit(mc0))
                return qk_sb, v_aug, units

            def emit_st_exp(g, qk_sb, t, hg, j_global):
                # ST = K@Q^T then exp, two 2-head sub-batches
                tsl = slice(t * 256, (t + 1) * 256)
                et4 = et_pool.tile([128, 4, 512], BF16, tag="et",
                                   name=f"et{j_global}")
                st2s = [ps_st.tile([128, 2, 512], F32, tag="st",
                                   name=f"st{j_global}_{sb}")
                        for sb in range(2)]
                # tile-major order (banks alternate within each tile): the
                # first exp's tile is fully written after 4 matmuls, not 6
                if ST_ORDER == "tile":
                    order = [(j, hh) for hh2 in (0, 2)
                             for j in range(2) for hh in (hh2, hh2 + 1)]
                else:
                    order = [(j, hh) for j in range(2) for hh in range(4)]
                for j, hh in order:
                    po = hh * 32
                    nc.tensor.matmul(
                        st2s[hh // 2][:, hh % 2, j * 256:(j + 1) * 256],
                        qk_sb[po:po + 32, 2 + hg,
                              t * 256 + j * 128:t * 256 + (j + 1) * 128],
                        qk_sb[po:po + 32, hg, tsl],
                        start=True, stop=True,
                        tile_position=(po, 0),
                    )
                for sb in range(2):
                    nc.scalar.activation(
                        et4[:, sb * 2:(sb + 1) * 2, :], st2s[sb], EXP
                    )
                return et4

            def emit_ctx(g, v_aug, t, hg, et4, j_global):
                # [ctx^T ; rowsum] accumulate: lhsT = [V_h | 1]
                direct = TAIL_DIRECT and j_global >= NIT - 2
                if direct:
                    # 33 rowsum copies; last iter's tile borrows from the
                    # draining ST pool so ps_acc never serializes the tail
                    pool = ps_st if j_global == NIT - 1 else ps_acc
                    tag = "st" if j_global == NIT - 1 else "c4"
                    acc4 = pool.tile([64, 4, 256], F32, tag=tag,
                                     name=f"acc{j_global}")
                else:
                    acc4 = ps_acc.tile([33, 4, 256], F32, tag="c4",
                                       name=f"acc{j_global}")
                ncol = 64 if direct else 33
                for hh in range(4):
                    h = hg * 4 + hh
                    for j in range(2):
                        nc.tensor.matmul(
                            acc4[:, hh, :],
                            v_aug[:, t * 2 + j, h, 0:ncol],
                            et4[:, hh, j * 256:(j + 1) * 256],
                            start=(j == 0), stop=(j == 1),
                        )
                return acc4

            def emit_direct_norm(j, acc4, g, t, hg):
                """Tail path: classic in-place reciprocal (scalar is idle
                once the score exps end) -- no DMA round trip."""
                tsl = slice(t * 256, (t + 1) * 256)
                ln4 = z_pool.tile([32, 4, 256], F32, tag="dln",
                                  name=f"dln{j}", bufs=2)
                rcp4 = z_pool.tile([32, 4, 256], F32, tag="drc",
                                   name=f"drc{j}", bufs=2)
                nc.scalar.activation(ln4, acc4[32:64, :, :], LOG)
                nc.scalar.activation(rcp4, ln4, EXP, scale=-1.0)
                ctxT_sb = ctxT_tiles[g]
                for hh in range(4):
                    nc.vector.tensor_tensor(
                        ctxT_sb[hh * 32:(hh + 1) * 32, hg, tsl],
                        acc4[0:32, hh, :], rcp4[:, hh, :], MUL,
                    )

            # ---- reciprocal dance stages (batched per iteration-pair) -----
            state = {}   # j_global -> dict with tiles + (g, t, hg)
            pstate = {}  # pair index p -> dict with zrow/rcp4 tiles

            def emit_copy_t1(j):
                st = state[j]
                p, half = j // 2, j % 2
                if half == 0:
                    cacc = cacc_pool.tile([33, 2, 4, 256], BF16, tag="cacc",
                                          name=f"cacc{p}")
                    pstate[p] = dict(cacc=cacc)
                cacc = pstate[p]["cacc"]
                nc.vector.tensor_copy(cacc[:, half, :, :], st["acc4"])
                st["cacc"] = cacc
                st["half"] = half
                st["acc4"] = None
                if half == 1:
                    zrow = z_pool.tile([128, 16], BF16, tag="zrow",
                                       name=f"zrow{p}")
                    # SP queue holds only T1/T3 dance triggers, so its
                    # blocking waits never stall a compute engine's stream
                    nc.sync.dma_start(out=zrow, in_=cacc[32:33, :, :, :])
                    pstate[p]["zrow"] = zrow

            def emit_recip_t2(p):
                ps = pstate[p]
                zl = z_pool.tile([128, 16], F32, tag="zl", name=f"zl{p}")
                zrcp = z_pool.tile([128, 16], BF16, tag="zrcp",
                                   name=f"zrcp{p}")
                nc.scalar.activation(zl, ps["zrow"], LOG)
                nc.scalar.activation(zrcp, zl, EXP, scale=-1.0)
                ps["zrow"] = None
                # trigger from the Act queue right after the exp: no wait
                nc.scalar.dma_start(out=rdz[p, :], in_=zrcp)
                ps["zrcp"] = zrcp

            def emit_t3(p):
                ps = pstate[p]
                rcp4 = rcp4_pool.tile([32, 2, 4, 256], BF16, tag="rcp4",
                                      name=f"rcp4_{p}")
                nc.sync.dma_start(out=rcp4,
                                  in_=rdz[p, :].partition_broadcast(32))
                ps["rcp4"] = rcp4

            def emit_mult(j, ctxT_tiles):
                st = state.pop(j)
                p, half = j // 2, j % 2
                g, t, hg = st["g"], st["t"], st["hg"]
                tsl = slice(t * 256, (t + 1) * 256)
                ctxT_sb = ctxT_tiles[g]
                rcp4 = pstate[p]["rcp4"]
                for hh in range(4):
                    # drain tail: split the serial Pool mult backlog onto
                    # the then-idle DVE (bf16 2x rate)
                    on_dve = (hh < MULT_DVE or
                              (j >= NIT - TAIL_DVE and hh % 2 == 0))
                    eng = nc.vector if on_dve else nc.gpsimd
                    if MULT_OP == "stt" and eng is nc.gpsimd:
                        eng.scalar_tensor_tensor(
                            ctxT_sb[hh * 32:(hh + 1) * 32, hg, tsl],
                            st["cacc"][0:32, half, hh, :], 1.0,
                            rcp4[:, half, hh, :],
                            mybir.AluOpType.bypass, MUL,
                        )
                    else:
                        eng.tensor_tensor(
                            ctxT_sb[hh * 32:(hh + 1) * 32, hg, tsl],
                            st["cacc"][0:32, half, hh, :],
                            rcp4[:, half, hh, :], MUL,
                        )
                if half == 1:
                    pstate.pop(p)

            def dance_pre(k, ctxT_tiles):
                """Stages for older iterations, run at iteration-slot k.
                Pair p's chain: T1 fires with copy(2p+1) at slot
                2p+1+CTX_LAG, recip+T2 one slot later, T3 the next, mults
                at 2p+MULT_LAG / 2p+1+MULT_LAG."""
                if k - MULT_LAG in state:
                    emit_mult(k - MULT_LAG, ctxT_tiles)
                if T3_SLOT > 0 and (k - 2 - CTX_LAG - T3_SLOT) >= 0 \
                        and (k - 2 - CTX_LAG - T3_SLOT) % 2 == 0:
                    p = (k - 2 - CTX_LAG - T3_SLOT) // 2
                    if p in pstate and "rcp4" not in pstate[p]:
                        emit_t3(p)
                if (k - 2 - CTX_LAG) >= 0 and (k - 2 - CTX_LAG) % 2 == 0:
                    p = (k - 2 - CTX_LAG) // 2
                    if p in pstate and "zrcp" not in pstate[p]:
                        emit_recip_t2(p)
                        if T3_SLOT == 0 or p >= NIT // 2 - TAIL_T3:
                            # tail pairs: T3 in the same slot (the SP queue
                            # is quiet there, and the shorter chain lets the
                            # final mults start a slot earlier)
                            emit_t3(p)

            def emit_c_units(g, ctxT_sb):
                o_sb = o_pool.tile([128, MC_G, 256], F32, tag="o",
                                   name=f"o{g}")

                def c_unit(mc0):
                    def _go():
                        ps = ps_misc.tile([128, 2, 256], F32, tag="ms",
                                          name=f"psc{g}_{mc0}")
                        for d in range(2):
                            mc = mc0 + d
                            for kc in range(2):
                                nc.tensor.matmul(
                                    ps[:, d, :],
                                    ctxT_sb[:, kc, mc * 128:(mc + 1) * 128],
                                    wl_sb[:, kc, :],
                                    start=(kc == 0), stop=(kc == 1),
                                )
                        for d in range(2):
                            mc = mc0 + d
                            nc.vector.tensor_tensor(
                                o_sb[:, mc, :], ps[:, d, :], bfin_sb, ADD
                            )
                        # ship this 256-point slice immediately: overlaps
                        # the store with remaining compute, shrinking the tail
                        rsl = slice(g * GP + mc0 * 128,
                                    g * GP + (mc0 + 2) * 128)
                        nc.sync.dma_start(
                            out=out[rsl, :].rearrange(
                                "(m p) n -> p m n", p=128),
                            in_=o_sb[:, mc0:mc0 + 2, :],
                        )
                    return _go

                return [c_unit(mc0) for mc0 in range(0, MC_G, 2)]

            def weave(c_units, a_units):
                """Readiness-greedy interleave.  c_unit[ci] becomes ready
                when the norm mults of track ci of the previous group have
                executed (slot 2*ci+MULT_LAG-8 of this group, + margin);
                a-units wait on the xg DMA issued at group start (A_DELAY
                slots), except the dependency-free memset."""
                if os.environ.get("WEAVE", "thr") == "greedy":
                    cand = []
                    for ci, u in enumerate(c_units):
                        rdy = max(0.0,
                                  2 * ci + MULT_LAG - 8 + 0.5 * C_MARGIN)
                        cand.append([rdy, 1, ci, u])
                    for ai, u in enumerate(a_units):
                        rdy = 0.0 if ai == 0 else A_DELAY
                        cand.append([rdy, 0, ai, u])
                    total = len(cand)
                    units = []
                    remaining = sorted(cand)
                    for s in range(total):
                        slot = 8.0 * s / total
                        pick = None
                        for ent in remaining:
                            if ent[0] <= slot:
                                pick = ent
                                break
                        if pick is None:
                            pick = remaining[0]
                        remaining.remove(pick)
                        units.append(pick[3])
                    return units
                total = len(c_units) + len(a_units)
                units = []
                ai = ci = 0
                for s in range(total):
                    thr = None
                    if ci < len(c_units):
                        thr = total * (2 * ci + MULT_LAG - 7
                                       + C_MARGIN) / 8.0
                    if thr is not None and (s >= thr or ai >= len(a_units)):
                        units.append(c_units[ci])
                        ci += 1
                    else:
                        units.append(a_units[ai])
                        ai += 1
                return units

            # ---- software-pipelined schedule ------------------------------
            xgs = {0: emit_load(0)}
            load_late_consts()
            qk0, va0, units0 = emit_a_units(0, *xgs.pop(0))
            for u in units0:
                u()
            if XG_EARLY:
                xgs[1] = emit_load(1)
            ab = {0: (qk0, va0)}
            ab_v = {}
            ctx_q = []
            ctxT_tiles = {}
            c_carry = []
            iters = [(t, hg) for t in range(TPG) for hg in (0, 1)]
            for g in range(NG):
                qk_sb, v_aug = ab.pop(g)
                ctxT_tiles[g] = ctx_pool.tile([128, 2, GP], BF16, tag="ctxT",
                                              name=f"ctxT{g}")
                a_units = []
                if g + 1 < NG:
                    if g + 1 not in xgs:
                        xgs[g + 1] = emit_load(g + 1)
                    qkn, van, aun = emit_a_units(g + 1, *xgs.pop(g + 1))
                    ab[g + 1] = (qkn, van)
                    a_units = aun
                units = weave(list(c_carry), a_units)
                c_carry = []
                ui = 0

                def pop_ctx(gq, entry):
                    pk, pt, phg, pet = entry
                    acc4 = emit_ctx(gq, ab_v[pk // 8], pt, phg, pet, pk)
                    if TAIL_DIRECT and pk >= NIT - 2:
                        emit_direct_norm(pk, acc4, gq, pt, phg)
                    else:
                        state[pk] = dict(acc4=acc4, g=gq, t=pt, hg=phg)
                        emit_copy_t1(pk)

                ab_v[g] = v_aug
                for i, (t, hg) in enumerate(iters):
                    k = g * 8 + i
                    dance_pre(k, ctxT_tiles)
                    if XG_EARLY and g + 2 < NG and 1 <= i <= 4:
                        # issue the g+2 input load a full group before its
                        # first consumer, in four slot-staggered chunks so
                        # the burst never crowds out the dance DMAs
                        if i == 1:
                            xgs[g + 2] = alloc_xg(g + 2)
                        load_xg_chunk(xgs[g + 2], g + 2, i - 1, 4)
                    take = (len(units) * (i + 1)) // len(iters) - ui
                    # half the units go before ST: PE filler while the ST
                    # tiles wait on the previous exp to release PSUM (but
                    # not at slot 0: there ST is the ready work)
                    pre = take // 2 if (i > 0 or PRE0) else 0
                    for _ in range(pre):
                        units[ui]()
                        ui += 1
                        take -= 1
                    et4 = emit_st_exp(g, qk_sb, t, hg, k)
                    while ctx_q and ctx_q[0][0] <= k - CTX_LAG:
                        pop_ctx(ctx_q[0][0] // 8, ctx_q.pop(0))
                    ctx_q.append((k, t, hg, et4))
                    for _ in range(take):
                        units[ui]()
                        ui += 1
                while ui < len(units):
                    units[ui]()
                    ui += 1
                # group-final: drain pending ctx before the boundary (the
                # ST pipeline restarts cleanly in the next group anyway)
                while ctx_q:
                    pop_ctx(ctx_q[0][0] // 8, ctx_q.pop(0))
                c_carry = emit_c_units(g, ctxT_tiles[g])

            # ---- epilogue: drain the dance, last group's C phase ----------
            k = NG * 8
            while state:
                dance_pre(k, ctxT_tiles)
                k += 1
            for u in c_carry:
                u()

    return nc


_PROG_CACHE = {}


def _get_program(masked: bool):
    if masked not in _PROG_CACHE:
        _PROG_CACHE[masked] = _build_program(masked)
    return _PROG_CACHE[masked]


def _prep_host(values, w_qkv, b_qkv, w_lin, b_lin):
    """Host-side weight preprocessing (all cheap, shared across cores)."""
    scale = 1.0 / np.sqrt(DH)
    w_qkv = np.asarray(w_qkv, np.float32).copy()
    b_qkv = np.asarray(b_qkv, np.float32).copy()
    w_lin = np.asarray(w_lin, np.float32)
    b_lin = np.asarray(b_lin, np.float32)
    w_qkv[:DOUT] *= scale
    b_qkv[:DOUT] *= scale

    wqk = np.ascontiguousarray(
        w_qkv[:2 * DOUT].T.reshape(2, 128, 512)
    )  # [k-chunk, k-part, row]
    wv = np.ascontiguousarray(w_qkv[2 * DOUT:].T.reshape(2, 128, 256))
    wl = np.ascontiguousarray(
        w_lin.T.reshape(2, 128, 256)).astype(ml_dtypes.bfloat16)
    bqk = np.ascontiguousarray(b_qkv[:2 * DOUT].reshape(4, 128).T)
    b_v = b_qkv[2 * DOUT:]  # unscaled: only the q section was scaled above
    bfin = (w_lin @ b_v + b_lin).astype(np.float32)
    return wqk, wv, wl, bqk, bfin


def _run(values_padded, mask, w_arrs, trace=False):
    """values_padded: [N, 256] in track-padded order; mask: None or [N]."""
    wqk, wv, wl, bqk, bfin = w_arrs
    masked = mask is not None
    nc = _get_program(masked)

    in_maps = []
    for c in range(N_CORES):
        sl = slice(c * PC, (c + 1) * PC)
        xTc = np.ascontiguousarray(
            values_padded[sl].T.reshape(2, 128, PC)
        )
        m = dict(xT=xTc, wqk=wqk, wv=wv, wl=wl, bqk=bqk, bfin=bfin)
        if masked:
            mc_ = np.ascontiguousarray(mask[sl], np.float32)
            m["mkf"] = mc_
            m["mkp"] = np.ascontiguousarray(mc_.reshape(PC // 128, 128).T)
        in_maps.append(m)

    if trace:
        _ensure_ntff_hook()
    res = run_bass_kernel_spmd(nc, in_maps, list(range(N_CORES)), trace=trace)
    outp = np.concatenate([res.results[c]["out"] for c in range(N_CORES)], 0)
    return outp, res


LAST_RESULTS = None


def kernel(values, w_qkv, b_qkv, w_lin, b_lin, track_ids, n_tracks,
           num_heads, _trace=False):
    global LAST_RESULTS
    values = np.asarray(values, np.float32)
    track_ids = np.asarray(track_ids, np.int32)
    n_tracks_i = int(n_tracks)
    num_heads_i = int(num_heads)
    assert values.shape == (N, DIN) and n_tracks_i == T and num_heads_i == H, (
        "kernel compiled for N=65536, d=256, T=256, H=8"
    )

    w_arrs = _prep_host(values, w_qkv, b_qkv, w_lin, b_lin)

    counts = np.bincount(track_ids, minlength=T)
    equal = bool((counts == L).all())

    if equal:
        outp, res = _run(values, None, w_arrs, trace=_trace)
        LAST_RESULTS = res
        return outp

    # general sorted-ragged path: scatter to padded [T, L] grid on host,
    # run the same device kernel with padding masked out of K and V, then
    # gather back (mirroring jax's oob-drop scatter / clip gather).
    starts = np.concatenate([[0], np.cumsum(counts)[:-1]])
    pos = np.arange(N, dtype=np.int64) - starts[track_ids]
    keep = pos < L
    rows = track_ids.astype(np.int64) * L + np.minimum(pos, L - 1)
    padded = np.zeros((T * L, DIN), np.float32)
    padded[rows[keep]] = values[keep]
    mask = np.zeros(T * L, np.float32)
    mask[rows[keep]] = 1.0
    outp, res = _run(padded, mask, w_arrs, trace=_trace)
    LAST_RESULTS = res
    return np.ascontiguousarray(outp[rows])
